# revision 5
# baseline (speedup 1.0000x reference)
"""GraphTransformer refiner on 8 Trainium2 NeuronCores.

Strategy (1D node-parallel, dst-sharded):
- Host: shard dst nodes across 8 cores; per core, sort local nodes by
  in-degree, tile 128 nodes; per tile pad slot count to a multiple of 4
  (uniform across cores so one SPMD program serves all 8). Slot index
  arrays + pad counts are precomputed; biases bk/bv/bskip fold away
  (softmax shift-invariance / alpha summing to 1); 1/sqrt(C) and bq fold
  into Wq/bq; bv/bskip/b_out fold into one output bias.
- Device per core: build a bf16 [k|v] table for ALL nodes (replicated
  projections, node-major rows via h-chunk-stationary matmuls), then for
  each dst tile gather kv rows per 4-slot group with indirect DMA,
  unnormalized segment softmax (scores are tiny, exp never overflows),
  slot accumulation via identity-matmul into PSUM, and a folded output
  projection producing out^T.
- Host: transpose, un-permute, concatenate.
"""

import numpy as np
import ml_dtypes

N, E, IN, HD, OUT, H, C = 50000, 800000, 128, 128, 32, 4, 32
NCORES = 8
SHARD = N // NCORES            # 6250
LT = 49                        # local node tiles (49*128 = 6272)
LPAD = LT * 128
NTAB = 98 * 512                # padded kv-table rows (50176)
DUMMY = N                      # zeroed dummy row for pad slots
BF16 = ml_dtypes.bfloat16


def _prep_edges(edge_index):
    src = np.asarray(edge_index[0], np.int64)
    dst = np.asarray(edge_index[1], np.int64)
    deg = np.bincount(dst, minlength=N)
    csr = np.zeros(N + 1, np.int64)
    np.cumsum(deg, out=csr[1:])
    order = np.argsort(dst, kind="stable")
    src_sorted = src[order]

    perms, degs_sorted = [], []
    for c in range(NCORES):
        ldeg = deg[c * SHARD:(c + 1) * SHARD]
        perm = np.argsort(-ldeg, kind="stable")
        perms.append(perm)
        d = np.zeros(LPAD, np.int64)
        d[:SHARD] = ldeg[perm]
        degs_sorted.append(d)

    # uniform per-tile slot counts across cores, padded to multiple of 4
    D = np.zeros(LT, np.int64)
    for t in range(LT):
        m = max(int(degs_sorted[c][t * 128:(t + 1) * 128].max())
                for c in range(NCORES))
        D[t] = max(4, ((m + 3) // 4) * 4)
    offs = np.zeros(LT + 1, np.int64)
    np.cumsum(D, out=offs[1:])
    S_total = int(offs[-1])

    idxs, padcs = [], []
    for c in range(NCORES):
        idx = np.full((128, S_total), DUMMY, np.int32)
        padc = np.zeros((128, LT), np.float32)
        perm = perms[c]
        for t in range(LT):
            Dt, o = int(D[t]), int(offs[t])
            for r in range(128):
                li = t * 128 + r
                if li < SHARD:
                    n = c * SHARD + int(perm[li])
                    d = int(deg[n])
                    if d:
                        idx[r, o:o + d] = src_sorted[csr[n]:csr[n] + d]
                    padc[r, t] = Dt - max(d, 1)
                else:
                    padc[r, t] = Dt - 1
        idxs.append(idx)
        padcs.append(padc)
    return perms, D, offs, S_total, idxs, padcs


def _build(S_total, D, offs):
    import concourse.bacc as bacc
    import concourse.bass as bass
    import concourse.mybir as mybir
    import concourse.tile as tile
    from concourse.masks import make_identity
    from concourse.vector_clock import ScopedClock

    # this walrus build rejects >1 sync wait on a Drain; split the
    # kernel-tail drain into a chain of single-wait drains
    def _drain_split(self, tick_clock, wait_clock):
        drain_inst = self.nc.sync.drain()
        wait_clock.add_sem_waits(
            drain_inst.ins, ScopedClock({None: tick_clock.global_clock}))
        si = drain_inst.ins.sync_info
        if si is not None and len(si.on_wait) > 1:
            waits = list(si.on_wait)
            drain_inst.ins.sync_info = mybir.SyncInfo(
                on_wait=waits[:1], on_update=list(si.on_update))
            for w in waits[1:]:
                d2 = self.nc.sync.drain()
                d2.ins.sync_info = mybir.SyncInfo(on_wait=[w], on_update=[])
        self.nc.all_engine_barrier()
        popped = self.nc._tile_sem_poison_stack.pop()
        assert popped is self._sem_poison
        self.nc.clear_and_free_semaphores(list(self.sems.allocated().values()))
        self.nc.all_engine_barrier()

    tile.TileContext._drain_and_barrier = _drain_split

    F32, BF, I32 = mybir.dt.float32, mybir.dt.bfloat16, mybir.dt.int32
    MUL, ADD, SUB = (mybir.AluOpType.mult, mybir.AluOpType.add,
                     mybir.AluOpType.subtract)
    X = mybir.AxisListType.X

    def view(ap, dims):
        return bass.AP(ap.tensor, ap.offset, [ap.ap[0]] + dims)

    nc = bacc.Bacc("TRN2", target_bir_lowering=False, debug=False,
                   num_devices=1)
    xt_d = nc.dram_tensor("xt", [128, NTAB], BF, kind="ExternalInput").ap()
    xl_d = nc.dram_tensor("xl", [128, LPAD], BF, kind="ExternalInput").ap()
    idx_d = nc.dram_tensor("idx", [128, S_total], I32,
                           kind="ExternalInput").ap()
    padc_d = nc.dram_tensor("padc", [128, LT], F32, kind="ExternalInput").ap()
    win_d = nc.dram_tensor("win", [128, 128], BF, kind="ExternalInput").ap()
    wkv_d = nc.dram_tensor("wkv", [128, 256], BF, kind="ExternalInput").ap()
    wq_d = nc.dram_tensor("wq", [128, 128], BF, kind="ExternalInput").ap()
    bq_d = nc.dram_tensor("bq", [1, 128], BF, kind="ExternalInput").ap()
    bin_d = nc.dram_tensor("bin", [128, 1], F32, kind="ExternalInput").ap()
    wout_d = nc.dram_tensor("wout", [128, OUT], BF, kind="ExternalInput").ap()
    wsk_d = nc.dram_tensor("wsk", [128, OUT], BF, kind="ExternalInput").ap()
    bfin_d = nc.dram_tensor("bfin", [OUT, 1], F32, kind="ExternalInput").ap()
    tab_d = nc.dram_tensor("kvtab", [NTAB, 256], BF, kind="Internal").ap()
    out_d = nc.dram_tensor("outT", [OUT, LPAD], F32,
                           kind="ExternalOutput").ap()

    with tile.TileContext(nc) as tc:
        with (
            tc.tile_pool(name="sb", bufs=1) as sb,
            tc.tile_pool(name="ps", bufs=1, space="PSUM") as ps,
        ):
            # constants
            win = sb.tile([128, 128], BF, tag="win")
            wkv = sb.tile([128, 256], BF, tag="wkv")
            wq = sb.tile([128, 128], BF, tag="wq")
            bq = sb.tile([1, 128], BF, tag="bq")
            bin_ = sb.tile([128, 1], F32, tag="bin")
            wout = sb.tile([128, OUT], BF, tag="wout")
            wsk = sb.tile([128, OUT], BF, tag="wsk")
            bfin = sb.tile([OUT, 1], F32, tag="bfin")
            ones = sb.tile([1, 128], BF, tag="ones")
            ident = sb.tile([128, 128], BF, tag="ident")
            zrow = sb.tile([1, 256], BF, tag="zrow")
            idx = sb.tile([128, S_total], I32, tag="idx")
            padc = sb.tile([128, LT], F32, tag="padc")
            hloc = sb.tile([128, LPAD], BF, tag="hloc")
            qsb = sb.tile([128, LPAD], BF, tag="qsb")
            for t_, d_ in ((win, win_d), (wkv, wkv_d), (wq, wq_d),
                           (bq, bq_d), (bin_, bin_d), (wout, wout_d),
                           (wsk, wsk_d), (bfin, bfin_d), (idx, idx_d),
                           (padc, padc_d)):
                nc.sync.dma_start(t_[:], d_[:])
            nc.gpsimd.memset(ones[:], 1.0)
            nc.gpsimd.memset(zrow[:], 0.0)
            make_identity(nc, ident[:])

            # phase A: local h^T and q (permuted order)
            for t in range(LT):
                s = slice(t * 128, (t + 1) * 128)
                xc = sb.tile([128, 128], BF, tag="xc")
                nc.sync.dma_start(xc[:], xl_d[:, s])
                hps = ps.tile([128, 512], mybir.dt.float32, tag="hb")
                nc.tensor.matmul(out=hps[:, :128], lhsT=win[:], rhs=xc[:],
                                 start=True, stop=True)
                nc.vector.tensor_scalar_add(hloc[:, s], hps[:, :128],
                                            bin_[:, 0:1])
                qps = ps.tile([128, 512], mybir.dt.float32, tag="kvps")
                nc.tensor.matmul(out=qps[:, :128], lhsT=hloc[:, s], rhs=wq[:],
                                 start=True, stop=False)
                nc.tensor.matmul(out=qps[:, :128], lhsT=ones[:], rhs=bq[:],
                                 start=False, stop=True)
                nc.scalar.copy(qsb[:, s], qps[:, :128])

            # phase B: kv table for all nodes
            for t in range(98):
                s = slice(t * 512, (t + 1) * 512)
                xb = sb.tile([128, 512], BF, tag="xb")
                nc.sync.dma_start(xb[:], xt_d[:, s])
                hps = ps.tile([128, 512], mybir.dt.float32, tag="hb")
                nc.tensor.matmul(out=hps[:], lhsT=win[:], rhs=xb[:],
                                 start=True, stop=True)
                hsb = sb.tile([128, 512], BF, tag="hsb")
                nc.vector.tensor_scalar_add(hsb[:], hps[:], bin_[:, 0:1])
                for c4 in range(4):
                    kvps = ps.tile([128, 512], mybir.dt.float32, tag="kvps")
                    nc.tensor.matmul(
                        out=kvps[:, :256],
                        lhsT=hsb[:, c4 * 128:(c4 + 1) * 128], rhs=wkv[:],
                        start=True, stop=True)
                    kvsb = sb.tile([128, 256], BF, tag="kvsb")
                    if c4 % 2:
                        nc.scalar.copy(kvsb[:], kvps[:, :256])
                    else:
                        nc.vector.tensor_copy(kvsb[:], kvps[:, :256])
                    r0 = t * 512 + c4 * 128
                    nc.sync.dma_start(tab_d[r0:r0 + 128, :], kvsb[:])
            nc.sync.dma_start(tab_d[DUMMY:DUMMY + 1, :], zrow[:])

            # phase C: edge attention per dst tile
            for t in range(LT):
                s = slice(t * 128, (t + 1) * 128)
                Dt, o = int(D[t]), int(offs[t])
                ngr = Dt // 4
                psA = ps.tile([128, 512], mybir.dt.float32, tag="hb")
                den = sb.tile([128, H], mybir.dt.float32, tag="den")
                for gi in range(ngr):
                    go = o + gi * 4
                    kvt = sb.tile([128, 4 * 256], BF, tag="kvt")
                    for sj in range(4):
                        nc.gpsimd.indirect_dma_start(
                            out=kvt[:, sj * 256:(sj + 1) * 256],
                            out_offset=None, in_=tab_d[:],
                            in_offset=bass.IndirectOffsetOnAxis(
                                ap=idx[:, go + sj:go + sj + 1], axis=0))
                    smul = sb.tile([128, 512], BF, tag="smul")
                    q_b = (qsb[:, s].rearrange("p (s f) -> p s f", s=1)
                           .to_broadcast([128, 4, 128]))
                    nc.vector.tensor_tensor(
                        out=smul[:].rearrange("p (s f) -> p s f", s=4),
                        in0=q_b, in1=view(kvt[:], [[256, 4], [1, 128]]),
                        op=MUL)
                    sc = sb.tile([128, 16], mybir.dt.float32, tag="sc")
                    nc.vector.tensor_reduce(
                        out=sc[:].rearrange("p (g o) -> p g o", o=1),
                        in_=smul[:].rearrange("p (g c) -> p g c", c=C),
                        axis=X, op=ADD)
                    pexp = sb.tile([128, 512], BF, tag="pexp")
                    sc_b = (sc[:].rearrange("p (g o) -> p g o", o=1)
                            .to_broadcast([128, 16, C]))
                    nc.scalar.activation(
                        out=pexp[:].rearrange("p (g c) -> p g c", c=C),
                        in_=sc_b, func=mybir.ActivationFunctionType.Exp)
                    if gi == 0:
                        nc.vector.tensor_reduce(
                            out=view(den[:], [[1, H], [0, 1]]),
                            in_=view(pexp[:], [[C, H], [128, 4]]),
                            axis=X, op=ADD)
                    else:
                        dtmp = sb.tile([128, H], mybir.dt.float32, tag="dtmp")
                        nc.vector.tensor_reduce(
                            out=view(dtmp[:], [[1, H], [0, 1]]),
                            in_=view(pexp[:], [[C, H], [128, 4]]),
                            axis=X, op=ADD)
                        nc.vector.tensor_tensor(out=den[:], in0=den[:],
                                                in1=dtmp[:], op=ADD)
                    pv = sb.tile([128, 512], BF, tag="pv")
                    nc.vector.tensor_tensor(
                        out=pv[:].rearrange("p (s f) -> p s f", s=4),
                        in0=pexp[:].rearrange("p (s f) -> p s f", s=4),
                        in1=view(bass.AP(kvt[:].tensor, kvt[:].offset + 128,
                                         kvt[:].ap), [[256, 4], [1, 128]]),
                        op=MUL)
                    nc.tensor.matmul(out=psA[:], lhsT=ident[:], rhs=pv[:],
                                     start=(gi == 0), stop=(gi == ngr - 1))
                acc = sb.tile([128, 128], mybir.dt.float32, tag="acc")
                nc.vector.tensor_reduce(
                    out=view(acc[:], [[1, 128], [0, 1]]),
                    in_=view(psA[:], [[1, 128], [128, 4]]), axis=X, op=ADD)
                den2 = sb.tile([128, H], mybir.dt.float32, tag="den2")
                nc.vector.tensor_tensor(
                    out=den2[:], in0=den[:],
                    in1=view(padc[:, t:t + 1], [[0, H]]), op=SUB)
                rden = sb.tile([128, H], mybir.dt.float32, tag="rden")
                nc.vector.reciprocal(rden[:], den2[:])
                zt = sb.tile([128, 128], BF, tag="zt")
                nc.vector.tensor_tensor(
                    out=zt[:].rearrange("p (h c) -> p h c", h=H),
                    in0=acc[:].rearrange("p (h c) -> p h c", h=H),
                    in1=view(rden[:], [[1, H], [0, C]]), op=MUL)
                ztp = ps.tile([128, 128], BF, tag="ztp")
                nc.tensor.transpose(out=ztp[:], in_=zt[:], identity=ident[:])
                ztsb = sb.tile([128, 128], BF, tag="ztsb")
                nc.scalar.copy(ztsb[:], ztp[:])
                ops = ps.tile([OUT, 128], mybir.dt.float32, tag="ops")
                nc.tensor.matmul(out=ops[:], lhsT=wout[:], rhs=ztsb[:],
                                 start=True, stop=False)
                nc.tensor.matmul(out=ops[:], lhsT=wsk[:], rhs=hloc[:, s],
                                 start=False, stop=True)
                osb = sb.tile([OUT, 128], mybir.dt.float32, tag="osb")
                nc.vector.tensor_scalar_add(osb[:], ops[:], bfin[:, 0:1])
                nc.sync.dma_start(out_d[:, s], osb[:])
    nc.compile()
    return nc


def _make_in_maps(inputs, x, perms, idxs, padcs):
    g = lambda k: np.asarray(inputs[k], np.float32)
    W_in, b_in, Wq, bq = g("W_in"), g("b_in"), g("Wq"), g("bq")
    Wk, Wv, bv = g("Wk"), g("Wv"), g("bv")
    Wskip, bskip, W_out, b_out = g("Wskip"), g("bskip"), g("W_out"), g("b_out")

    scale = np.float32(1.0 / np.sqrt(C))
    xt = np.zeros((128, NTAB), BF16)
    xt[:, :N] = x.T.astype(BF16)
    win = W_in.astype(BF16)
    wkv = np.concatenate([Wk, Wv], 1).astype(BF16)
    wq = (Wq * scale).astype(BF16)
    bqs = (bq * scale).reshape(1, HD).astype(BF16)
    binp = b_in.reshape(128, 1)
    woutb = W_out.astype(BF16)
    wskb = (Wskip @ W_out).astype(BF16)
    bfin = (b_out + bv @ W_out + bskip @ W_out).reshape(OUT, 1).astype(
        np.float32)

    in_maps = []
    for c in range(NCORES):
        xl = np.zeros((128, LPAD), BF16)
        blk = x[c * SHARD:(c + 1) * SHARD][perms[c]]
        xl[:, :SHARD] = blk.T.astype(BF16)
        in_maps.append({
            "xt": xt, "xl": xl, "idx": idxs[c], "padc": padcs[c],
            "win": win, "wkv": wkv, "wq": wq, "bq": bqs, "bin": binp,
            "wout": woutb, "wsk": wskb, "bfin": bfin,
        })
    return in_maps


def kernel(x, edge_index, W_in, b_in, Wq, bq, Wk, bk, Wv, bv, Wskip, bskip,
           W_out, b_out):
    x = np.asarray(x, np.float32)
    perms, D, offs, S_total, idxs, padcs = _prep_edges(edge_index)
    inputs = dict(W_in=W_in, b_in=b_in, Wq=Wq, bq=bq, Wk=Wk, Wv=Wv, bv=bv,
                  Wskip=Wskip, bskip=bskip, W_out=W_out, b_out=b_out)
    in_maps = _make_in_maps(inputs, x, perms, idxs, padcs)

    nc = _build(S_total, D, offs)
    from concourse import bass_utils
    res = bass_utils.run_bass_kernel_spmd(nc, in_maps,
                                          core_ids=list(range(NCORES)))
    out = np.empty((N, OUT), np.float32)
    for c in range(NCORES):
        oT = res.results[c]["outT"]
        out[c * SHARD + perms[c]] = oT[:, :SHARD].T
    return out


# revision 8
# speedup vs baseline: 1.1435x; 1.1435x over previous
"""GraphTransformer refiner on 8 Trainium2 NeuronCores.

Strategy (1D node-parallel, dst-sharded):
- Host: shard dst nodes across 8 cores; per core, sort local nodes by
  in-degree, tile 128 nodes; per tile pad slot count to a multiple of 4
  (uniform across cores so one SPMD program serves all 8). Slot index
  arrays + pad counts are precomputed; biases bk/bv/bskip fold away
  (softmax shift-invariance / alpha summing to 1); 1/sqrt(C) and bq fold
  into Wq/bq; bv/bskip/b_out fold into one output bias.
- Device per core: build a bf16 [k|v] table for ALL nodes (replicated
  projections, node-major rows via h-chunk-stationary matmuls), then for
  each dst tile gather kv rows per 4-slot group with indirect DMA,
  unnormalized segment softmax (scores are tiny, exp never overflows),
  slot accumulation via identity-matmul into PSUM, and a folded output
  projection producing out^T.
- Host: transpose, un-permute, concatenate.
"""

import numpy as np
import ml_dtypes

N, E, IN, HD, OUT, H, C = 50000, 800000, 128, 128, 32, 4, 32
NCORES = 8
SHARD = N // NCORES            # 6250
LT = 49                        # local node tiles (49*128 = 6272)
LPAD = LT * 128
NTAB = 98 * 512                # padded kv-table rows (50176)
DUMMY = N                      # zeroed dummy row for pad slots
BF16 = ml_dtypes.bfloat16


def _prep_edges(edge_index):
    src = np.asarray(edge_index[0], np.int64)
    dst = np.asarray(edge_index[1], np.int64)
    deg = np.bincount(dst, minlength=N)
    csr = np.zeros(N + 1, np.int64)
    np.cumsum(deg, out=csr[1:])
    order = np.argsort(dst, kind="stable")
    src_sorted = src[order]

    perms, degs_sorted = [], []
    for c in range(NCORES):
        ldeg = deg[c * SHARD:(c + 1) * SHARD]
        perm = np.argsort(-ldeg, kind="stable")
        perms.append(perm)
        d = np.zeros(LPAD, np.int64)
        d[:SHARD] = ldeg[perm]
        degs_sorted.append(d)

    # uniform per-tile slot counts across cores, padded to multiple of 4
    D = np.zeros(LT, np.int64)
    for t in range(LT):
        m = max(int(degs_sorted[c][t * 128:(t + 1) * 128].max())
                for c in range(NCORES))
        D[t] = max(4, ((m + 3) // 4) * 4)
    offs = np.zeros(LT + 1, np.int64)
    np.cumsum(D, out=offs[1:])
    S_total = int(offs[-1])

    idxs, padcs = [], []
    for c in range(NCORES):
        idx = np.full((128, S_total), DUMMY, np.int32)
        padc = np.zeros((128, LT), np.float32)
        perm = perms[c]
        for t in range(LT):
            Dt, o = int(D[t]), int(offs[t])
            for r in range(128):
                li = t * 128 + r
                if li < SHARD:
                    n = c * SHARD + int(perm[li])
                    d = int(deg[n])
                    if d:
                        idx[r, o:o + d] = src_sorted[csr[n]:csr[n] + d]
                    padc[r, t] = Dt - max(d, 1)
                else:
                    padc[r, t] = Dt - 1
        idxs.append(idx)
        padcs.append(padc)
    return perms, D, offs, S_total, idxs, padcs


def _build(S_total, D, offs):
    import concourse.bacc as bacc
    import concourse.bass as bass
    import concourse.mybir as mybir
    import concourse.tile as tile
    from concourse.masks import make_identity
    from concourse.vector_clock import ScopedClock

    # this walrus build rejects >1 sync wait on a Drain; split the
    # kernel-tail drain into a chain of single-wait drains
    def _drain_split(self, tick_clock, wait_clock):
        drain_inst = self.nc.sync.drain()
        wait_clock.add_sem_waits(
            drain_inst.ins, ScopedClock({None: tick_clock.global_clock}))
        si = drain_inst.ins.sync_info
        if si is not None and len(si.on_wait) > 1:
            waits = list(si.on_wait)
            drain_inst.ins.sync_info = mybir.SyncInfo(
                on_wait=waits[:1], on_update=list(si.on_update))
            for w in waits[1:]:
                d2 = self.nc.sync.drain()
                d2.ins.sync_info = mybir.SyncInfo(on_wait=[w], on_update=[])
        self.nc.all_engine_barrier()
        popped = self.nc._tile_sem_poison_stack.pop()
        assert popped is self._sem_poison
        self.nc.clear_and_free_semaphores(list(self.sems.allocated().values()))
        self.nc.all_engine_barrier()

    tile.TileContext._drain_and_barrier = _drain_split

    F32, BF, I32 = mybir.dt.float32, mybir.dt.bfloat16, mybir.dt.int32
    MUL, ADD, SUB = (mybir.AluOpType.mult, mybir.AluOpType.add,
                     mybir.AluOpType.subtract)
    X = mybir.AxisListType.X

    def view(ap, dims):
        return bass.AP(ap.tensor, ap.offset, [ap.ap[0]] + dims)

    nc = bacc.Bacc("TRN2", target_bir_lowering=False, debug=False,
                   num_devices=1, num_swdge_queues=4)
    xt_d = nc.dram_tensor("xt", [128, NTAB], BF, kind="ExternalInput").ap()
    xl_d = nc.dram_tensor("xl", [128, LPAD], BF, kind="ExternalInput").ap()
    idx_d = nc.dram_tensor("idx", [128, S_total], I32,
                           kind="ExternalInput").ap()
    padc_d = nc.dram_tensor("padc", [128, LT], F32, kind="ExternalInput").ap()
    win_d = nc.dram_tensor("win", [128, 128], BF, kind="ExternalInput").ap()
    wkv_d = nc.dram_tensor("wkv", [128, 256], BF, kind="ExternalInput").ap()
    wq_d = nc.dram_tensor("wq", [128, 128], BF, kind="ExternalInput").ap()
    bq_d = nc.dram_tensor("bq", [1, 128], BF, kind="ExternalInput").ap()
    bin_d = nc.dram_tensor("bin", [128, 1], F32, kind="ExternalInput").ap()
    wout_d = nc.dram_tensor("wout", [128, OUT], BF, kind="ExternalInput").ap()
    wsk_d = nc.dram_tensor("wsk", [128, OUT], BF, kind="ExternalInput").ap()
    bfin_d = nc.dram_tensor("bfin", [OUT, 1], F32, kind="ExternalInput").ap()
    tab_d = nc.dram_tensor("kvtab", [NTAB, 256], BF, kind="Internal").ap()
    out_d = nc.dram_tensor("outT", [OUT, LPAD], F32,
                           kind="ExternalOutput").ap()

    with tile.TileContext(nc) as tc:
        with (
            tc.tile_pool(name="sb", bufs=1) as sb,
            tc.tile_pool(name="sb3", bufs=3) as sb3,
            tc.tile_pool(name="kvp", bufs=6) as kvp,
            tc.tile_pool(name="ps", bufs=2, space="PSUM") as ps,
        ):
            # constants
            win = sb.tile([128, 128], BF, tag="win")
            wkv = sb.tile([128, 256], BF, tag="wkv")
            wq = sb.tile([128, 128], BF, tag="wq")
            bq = sb.tile([1, 128], BF, tag="bq")
            bin_ = sb.tile([128, 1], F32, tag="bin")
            wout = sb.tile([128, OUT], BF, tag="wout")
            wsk = sb.tile([128, OUT], BF, tag="wsk")
            bfin = sb.tile([OUT, 1], F32, tag="bfin")
            ones = sb.tile([1, 128], BF, tag="ones")
            ident = sb.tile([128, 128], BF, tag="ident")
            zrow = sb.tile([1, 256], BF, tag="zrow")
            idx = sb.tile([128, S_total], I32, tag="idx")
            padc = sb.tile([128, LT], F32, tag="padc")
            hloc = sb.tile([128, LPAD], BF, tag="hloc")
            qsb = sb.tile([128, LPAD], BF, tag="qsb")
            for t_, d_ in ((win, win_d), (wkv, wkv_d), (wq, wq_d),
                           (bq, bq_d), (bin_, bin_d), (wout, wout_d),
                           (wsk, wsk_d), (bfin, bfin_d), (idx, idx_d),
                           (padc, padc_d)):
                nc.sync.dma_start(t_[:], d_[:])
            nc.gpsimd.memset(ones[:], 1.0)
            nc.gpsimd.memset(zrow[:], 0.0)
            make_identity(nc, ident[:])

            # phase A: local h^T and q (permuted order)
            for t in range(LT):
                s = slice(t * 128, (t + 1) * 128)
                xc = sb3.tile([128, 128], BF, tag="xc")
                nc.sync.dma_start(xc[:], xl_d[:, s])
                hps = ps.tile([128, 512], mybir.dt.float32, tag="hb")
                nc.tensor.matmul(out=hps[:, :128], lhsT=win[:], rhs=xc[:],
                                 start=True, stop=True)
                nc.vector.tensor_scalar_add(hloc[:, s], hps[:, :128],
                                            bin_[:, 0:1])
                qps = ps.tile([128, 512], mybir.dt.float32, tag="kvps")
                nc.tensor.matmul(out=qps[:, :128], lhsT=hloc[:, s], rhs=wq[:],
                                 start=True, stop=False)
                nc.tensor.matmul(out=qps[:, :128], lhsT=ones[:], rhs=bq[:],
                                 start=False, stop=True)
                nc.scalar.copy(qsb[:, s], qps[:, :128])

            # phase B: kv table for all nodes
            for t in range(98):
                s = slice(t * 512, (t + 1) * 512)
                xb = sb3.tile([128, 512], BF, tag="xb")
                nc.sync.dma_start(xb[:], xt_d[:, s])
                hps = ps.tile([128, 512], mybir.dt.float32, tag="hb")
                nc.tensor.matmul(out=hps[:], lhsT=win[:], rhs=xb[:],
                                 start=True, stop=True)
                hsb = sb3.tile([128, 512], BF, tag="hsb")
                nc.vector.tensor_scalar_add(hsb[:], hps[:], bin_[:, 0:1])
                for c4 in range(4):
                    kvps = ps.tile([128, 512], mybir.dt.float32, tag="kvps")
                    nc.tensor.matmul(
                        out=kvps[:, :256],
                        lhsT=hsb[:, c4 * 128:(c4 + 1) * 128], rhs=wkv[:],
                        start=True, stop=True)
                    kvsb = sb3.tile([128, 256], BF, tag="kvsb")
                    if c4 % 2:
                        nc.scalar.copy(kvsb[:], kvps[:, :256])
                    else:
                        nc.vector.tensor_copy(kvsb[:], kvps[:, :256])
                    r0 = t * 512 + c4 * 128
                    nc.sync.dma_start(tab_d[r0:r0 + 128, :], kvsb[:])
            nc.sync.dma_start(tab_d[DUMMY:DUMMY + 1, :], zrow[:])

            # phase C: edge attention per dst tile
            for t in range(LT):
                s = slice(t * 128, (t + 1) * 128)
                Dt, o = int(D[t]), int(offs[t])
                ngr = Dt // 4
                psA = ps.tile([128, 512], mybir.dt.float32, tag="hb")
                den = sb3.tile([128, H], mybir.dt.float32, tag="den")
                for gi in range(ngr):
                    go = o + gi * 4
                    kvt = kvp.tile([128, 4 * 256], BF, tag="kvt")
                    for sj in range(4):
                        gin = nc.gpsimd.indirect_dma_start(
                            out=kvt[:, sj * 256:(sj + 1) * 256],
                            out_offset=None, in_=tab_d[:],
                            in_offset=bass.IndirectOffsetOnAxis(
                                ap=idx[:, go + sj:go + sj + 1], axis=0))
                        gin.ins.queue = f"qPoolDynamic{sj or ''}"
                    smul = sb3.tile([128, 512], BF, tag="smul")
                    q_b = (qsb[:, s].rearrange("p (s f) -> p s f", s=1)
                           .to_broadcast([128, 4, 128]))
                    nc.vector.tensor_tensor(
                        out=smul[:].rearrange("p (s f) -> p s f", s=4),
                        in0=q_b, in1=view(kvt[:], [[256, 4], [1, 128]]),
                        op=MUL)
                    sc = sb3.tile([128, 16], mybir.dt.float32, tag="sc")
                    nc.vector.tensor_reduce(
                        out=sc[:].rearrange("p (g o) -> p g o", o=1),
                        in_=smul[:].rearrange("p (g c) -> p g c", c=C),
                        axis=X, op=ADD)
                    pexp = sb3.tile([128, 512], BF, tag="pexp")
                    sc_b = (sc[:].rearrange("p (g o) -> p g o", o=1)
                            .to_broadcast([128, 16, C]))
                    nc.scalar.activation(
                        out=pexp[:].rearrange("p (g c) -> p g c", c=C),
                        in_=sc_b, func=mybir.ActivationFunctionType.Exp)
                    if gi == 0:
                        nc.vector.tensor_reduce(
                            out=view(den[:], [[1, H], [0, 1]]),
                            in_=view(pexp[:], [[C, H], [128, 4]]),
                            axis=X, op=ADD)
                    else:
                        dtmp = sb3.tile([128, H], mybir.dt.float32, tag="dtmp")
                        nc.vector.tensor_reduce(
                            out=view(dtmp[:], [[1, H], [0, 1]]),
                            in_=view(pexp[:], [[C, H], [128, 4]]),
                            axis=X, op=ADD)
                        nc.vector.tensor_tensor(out=den[:], in0=den[:],
                                                in1=dtmp[:], op=ADD)
                    pv = sb3.tile([128, 512], BF, tag="pv")
                    nc.vector.tensor_tensor(
                        out=pv[:].rearrange("p (s f) -> p s f", s=4),
                        in0=pexp[:].rearrange("p (s f) -> p s f", s=4),
                        in1=view(bass.AP(kvt[:].tensor, kvt[:].offset + 128,
                                         kvt[:].ap), [[256, 4], [1, 128]]),
                        op=MUL)
                    nc.tensor.matmul(out=psA[:], lhsT=ident[:], rhs=pv[:],
                                     start=(gi == 0), stop=(gi == ngr - 1))
                acc = sb3.tile([128, 128], mybir.dt.float32, tag="acc")
                nc.vector.tensor_reduce(
                    out=view(acc[:], [[1, 128], [0, 1]]),
                    in_=view(psA[:], [[1, 128], [128, 4]]), axis=X, op=ADD)
                den2 = sb3.tile([128, H], mybir.dt.float32, tag="den2")
                nc.vector.tensor_tensor(
                    out=den2[:], in0=den[:],
                    in1=view(padc[:, t:t + 1], [[0, H]]), op=SUB)
                rden = sb3.tile([128, H], mybir.dt.float32, tag="rden")
                nc.vector.reciprocal(rden[:], den2[:])
                zt = sb3.tile([128, 128], BF, tag="zt")
                nc.vector.tensor_tensor(
                    out=zt[:].rearrange("p (h c) -> p h c", h=H),
                    in0=acc[:].rearrange("p (h c) -> p h c", h=H),
                    in1=view(rden[:], [[1, H], [0, C]]), op=MUL)
                ztp = ps.tile([128, 128], BF, tag="ztp")
                nc.tensor.transpose(out=ztp[:], in_=zt[:], identity=ident[:])
                ztsb = sb3.tile([128, 128], BF, tag="ztsb")
                nc.scalar.copy(ztsb[:], ztp[:])
                ops = ps.tile([OUT, 128], mybir.dt.float32, tag="ops")
                nc.tensor.matmul(out=ops[:], lhsT=wout[:], rhs=ztsb[:],
                                 start=True, stop=False)
                nc.tensor.matmul(out=ops[:], lhsT=wsk[:], rhs=hloc[:, s],
                                 start=False, stop=True)
                osb = sb3.tile([OUT, 128], mybir.dt.float32, tag="osb")
                nc.vector.tensor_scalar_add(osb[:], ops[:], bfin[:, 0:1])
                nc.sync.dma_start(out_d[:, s], osb[:])
    nc.compile()
    return nc


def _make_in_maps(inputs, x, perms, idxs, padcs):
    g = lambda k: np.asarray(inputs[k], np.float32)
    W_in, b_in, Wq, bq = g("W_in"), g("b_in"), g("Wq"), g("bq")
    Wk, Wv, bv = g("Wk"), g("Wv"), g("bv")
    Wskip, bskip, W_out, b_out = g("Wskip"), g("bskip"), g("W_out"), g("b_out")

    scale = np.float32(1.0 / np.sqrt(C))
    xt = np.zeros((128, NTAB), BF16)
    xt[:, :N] = x.T.astype(BF16)
    win = W_in.astype(BF16)
    wkv = np.concatenate([Wk, Wv], 1).astype(BF16)
    wq = (Wq * scale).astype(BF16)
    bqs = (bq * scale).reshape(1, HD).astype(BF16)
    binp = b_in.reshape(128, 1)
    woutb = W_out.astype(BF16)
    wskb = (Wskip @ W_out).astype(BF16)
    bfin = (b_out + bv @ W_out + bskip @ W_out).reshape(OUT, 1).astype(
        np.float32)

    in_maps = []
    for c in range(NCORES):
        xl = np.zeros((128, LPAD), BF16)
        blk = x[c * SHARD:(c + 1) * SHARD][perms[c]]
        xl[:, :SHARD] = blk.T.astype(BF16)
        in_maps.append({
            "xt": xt, "xl": xl, "idx": idxs[c], "padc": padcs[c],
            "win": win, "wkv": wkv, "wq": wq, "bq": bqs, "bin": binp,
            "wout": woutb, "wsk": wskb, "bfin": bfin,
        })
    return in_maps


def kernel(x, edge_index, W_in, b_in, Wq, bq, Wk, bk, Wv, bv, Wskip, bskip,
           W_out, b_out):
    x = np.asarray(x, np.float32)
    perms, D, offs, S_total, idxs, padcs = _prep_edges(edge_index)
    inputs = dict(W_in=W_in, b_in=b_in, Wq=Wq, bq=bq, Wk=Wk, Wv=Wv, bv=bv,
                  Wskip=Wskip, bskip=bskip, W_out=W_out, b_out=b_out)
    in_maps = _make_in_maps(inputs, x, perms, idxs, padcs)

    nc = _build(S_total, D, offs)
    from concourse import bass_utils
    res = bass_utils.run_bass_kernel_spmd(nc, in_maps,
                                          core_ids=list(range(NCORES)))
    out = np.empty((N, OUT), np.float32)
    for c in range(NCORES):
        oT = res.results[c]["outT"]
        out[c * SHARD + perms[c]] = oT[:, :SHARD].T
    return out
